# revision 74
# baseline (speedup 1.0000x reference)
"""Trainium2 Bass kernel for nn_AsaTgcn (typed-GCN with concat-attention).

Math (per batch element, L=128 tokens, D=256, NT=47 dep types):
  de[i,j,:] = E'[v[i,j]]  where E' = dep_emb with row 0 zeroed, v = dep_value
  score[i,j] = (seq_i . seq_j + de[i,j] . de[j,i]) / sqrt(D)
  att = softmax(score, -1) * dep_adj
  out[i] = sum_j att[i,j] (seq_j @ W) + sum_j att[i,j] (de[j,i] @ W) + b

Key algebraic reductions (avoid the [L,L,D] de tensor entirely):
  de[i,j] . de[j,i]   = G'[v[i,j], v[j,i]],  G' = E' E'^T  (47x47 Gram table)
  sum_j att[i,j] de[j,i]@W = C @ (E'W),  C[i,t] = sum_j att[i,j]*[v[j,i]==t]

v4 structure:
  - inputs host-packed into two [128, C] blobs + one row blob (3 DMAs instead
    of 20: HWDGE issue overhead is 625ns each and strictly serial).
  - the gram-table chain (emb -> E'^T -> G' -> DRAM roundtrip -> replicated
    gather table) is emitted first under tc.high_priority(): it gates score2
    and therefore layer 0.
  - score2 extraction: gather + km mask (bf16) + 2x-mode binary tree over the
    16-wrap axis.
  - one-hot st3[i, t, j] built once by a single DVE is_equal (fills the
    DVE idle window while the gram DMAs fly); per-layer C = one bf16
    broadcast-multiply + in-place halving tree + final 8-wide reduce.
  - fp32r (full fp32 via rounded operands, 1 PE cycle/row at >=256 moving
    cols) for tw / out1 / out2 / E'W / bias matmuls; fp32 kept for the
    score-critical s1 contraction inputs (seqT rounded once, shared).
  - bias b_l applied as a K=1 outer-product matmul accumulated into out_ps.

Sharding: pure data parallel, batch element b -> NeuronCore b (B == 8).
"""

import os

import numpy as np

import concourse.bass as bass
import concourse.mybir as mybir
import concourse.tile as tile
from concourse import bacc
from concourse.bass_utils import run_bass_kernel_spmd
from concourse.masks import make_identity

dt = mybir.dt
Alu = mybir.AluOpType
Act = mybir.ActivationFunctionType
Axis = mybir.AxisListType

B, L, D, NT, R = 8, 128, 256, 47, 64
EPS = 1e-3
BN_SCALE = float(1.0 / np.sqrt(1.0 + EPS))
INV_SQRT_D = float(1.0 / np.sqrt(D))
KD = D // 128
NT2 = NT * NT
NTC = NT - 1  # C-pipeline t range [1,47): t=0 contributes nothing

KSTOP = int(os.environ.get("KSTOP", "99"))


def _build_graph(nc: bass.Bass, tc: tile.TileContext):
    f32 = dt.float32
    f32r = dt.float32r
    bf16 = dt.bfloat16

    # blob_m [128, 256]: emb (row0 zeroed, zero-padded) — first, it gates the
    #   gram-table chain
    # blob_e [128, 513]: depv(0:128, int32 bits) | mask(128:129, int32 bits) |
    #   text(129:385) | adj(385:513)
    # blob_b [128, 1664] (f32r): W1/W2/W3 ki-ko-n | fcw ki-ko-n
    # blob_r [1, 1347]: gamma|beta|b1|b2|b3|ens|fcb
    blobm_d = nc.declare_dram_parameter("blob_m", [128, 256], f32, isOutput=False)
    blobe_d = nc.declare_dram_parameter("blob_e", [128, 513], f32, isOutput=False)
    blobb_d = nc.declare_dram_parameter("blob_b", [128, 1664], f32r, isOutput=False)
    blobr_d = nc.declare_dram_parameter("blob_r", [1, 1347], f32, isOutput=False)
    out_d = nc.declare_dram_parameter("out", [1, R], f32, isOutput=True)

    gflat_dram = nc.dram_tensor("gflat_scratch", [NT, NT], f32)

    cpool = tc.alloc_tile_pool(name="const", bufs=1)
    wpool = tc.alloc_tile_pool(name="work", bufs=3)
    pst = tc.alloc_tile_pool(name="ps_t", bufs=2, space="PSUM")
    psm = tc.alloc_tile_pool(name="ps_mm", bufs=1, space="PSUM")
    psa = tc.alloc_tile_pool(name="ps_acc", bufs=1, space="PSUM")

    def _stop(stage, src_ap):
        if KSTOP != stage:
            return False
        nc.sync.dma_start(out_d.ap(), src_ap)
        for p in (psa, psm, pst, wpool, cpool):
            p.release()
        return True

    # ---------------- blob DMAs ----------------
    blobm = cpool.tile([128, 256], f32, tag="blobm")
    nc.sync.dma_start(blobm[:], blobm_d.ap())
    blobe = cpool.tile([128, 513], f32, tag="blobe")
    nc.sync.dma_start(blobe[:], blobe_d.ap())
    blobr = cpool.tile([1, 1347], f32, tag="blobr")
    nc.sync.dma_start(blobr[:], blobr_d.ap())
    blobb = cpool.tile([128, 1664], f32r, tag="blobb")

    emb_sb = blobm[:, 0:256]
    v_i = blobe[:, 0:128].bitcast(dt.int32)
    m_i = blobe[:, 128:129].bitcast(dt.int32)
    text_sb = blobe[:, 129:385]
    adj_sb = blobe[:, 385:513]
    gb_row = blobr[0:1, 0:512]
    ens_sb = blobr[0:1, 1280:1283]
    fcb_sb = blobr[0:1, 1283:1347]
    W_sb = [blobb[:, 512 * l : 512 * (l + 1)].rearrange("p (k n) -> p k n", k=KD) for l in range(3)]
    fcw_sb = blobb[:, 1536:1664].bitcast(f32).rearrange("p (k n) -> p k n", k=KD)

    # ---------------- gram-table chain (critical path to score2) ----------
    ident = cpool.tile([128, 128], f32, tag="ident")
    with tc.high_priority():
        make_identity(nc, ident[:])
        et_sb = cpool.tile([128, KD, NT], f32r, tag="et")
        et_f = cpool.tile([128, KD, NT], f32, tag="et_f")
        for k in range(KD):
            tp = pst.tile([128, 128], f32, tag="tps")
            nc.tensor.transpose(tp[:], emb_sb[:, k * 128 : (k + 1) * 128], ident[:])
            nc.vector.tensor_copy(et_sb[:, k, :], tp[:, 0:NT])
            nc.vector.tensor_copy(et_f[:, k, :], tp[:, 0:NT])
        g_ps = psm.tile([NT, NT], f32, tag="mm_small")
        for k in range(KD):
            nc.tensor.matmul(g_ps[:], et_f[:, k, :], et_f[:, k, :], start=(k == 0), stop=(k == KD - 1))
        g_sb = cpool.tile([NT, NT], f32, tag="g_sb")
        nc.vector.tensor_scalar(g_sb[:], g_ps[:], INV_SQRT_D, None, Alu.mult)
        # Act HWDGE queue: roundtrip first, then the (late-needed) weight blob
        nc.scalar.dma_start(gflat_dram.ap(), g_sb[:])
        gtab = cpool.tile([128, NT2], f32, tag="gtab")
        nc.scalar.dma_start(gtab[:], bass.AP(gflat_dram, 0, [[0, 128], [1, NT2]]))
    # weight blob is only needed at the first tw matmul (~22us in): hold its
    # transfer off the DMA engines until the gram-table replicate has flown
    # (tuned: 15us clears the gtab transfer tail; W still lands ~5us early)
    with tc.tile_wait_until(0.017):
        nc.scalar.dma_start(blobb[:], blobb_d.ap())

    # ---------------- constants ----------------
    ident_bf = cpool.tile([128, 128], bf16, tag="ident_bf")
    nc.vector.tensor_copy(ident_bf[:], ident[:])
    ones_col = cpool.tile([1, 128], f32, tag="ones_col")
    nc.gpsimd.memset(ones_col[:], 1.0)
    ones_col_r = cpool.tile([1, 128], f32r, tag="ones_col_r")
    nc.scalar.copy(ones_col_r[:], ones_col[:])

    iota_i = cpool.tile([128, NT], dt.int32, tag="iota_i")
    nc.gpsimd.iota(iota_i[:], pattern=[[1, NT]], base=0, channel_multiplier=0)
    iota_bf = cpool.tile([128, NT], bf16, tag="iota_bf")
    nc.vector.tensor_copy(iota_bf[:], iota_i[:])

    # km[p, k] = (p % 16 == k)
    pm_i = cpool.tile([128, 16], dt.int32, tag="pm_i")
    nc.gpsimd.iota(pm_i[:], pattern=[[0, 16]], base=0, channel_multiplier=1)
    pm16_i = cpool.tile([128, 16], dt.int32, tag="pm16_i")
    nc.vector.tensor_scalar(pm16_i[:], pm_i[:], 15, None, Alu.bitwise_and)
    km_bf = cpool.tile([128, 16], bf16, tag="km_bf")
    nc.vector.tensor_tensor(km_bf[:], pm16_i[:], iota_i[:, 0:16], Alu.is_equal)

    # ---------------- keys / vT / bn / seqT ----------------
    v_f = cpool.tile([L, L], f32, tag="v_f")
    nc.vector.tensor_copy(v_f[:], v_i[:])
    vT_ps = pst.tile([128, 128], f32, tag="tps")
    nc.tensor.transpose(vT_ps[:], v_f[:], ident[:])
    vT_f = cpool.tile([L, L], f32, tag="vT_f")
    nc.vector.tensor_copy(vT_f[:], vT_ps[:])
    vT_bf = cpool.tile([L, L], bf16, tag="vT_bf")
    nc.vector.tensor_copy(vT_bf[:], vT_f[:])

    key_f = wpool.tile([L, L], f32, tag="key_f")
    nc.vector.scalar_tensor_tensor(key_f[:], v_f[:], float(NT), vT_f[:], Alu.mult, Alu.add)
    idx_sb = cpool.tile([L, L], dt.int16, tag="idx")
    nc.vector.tensor_copy(idx_sb[:], key_f[:])

    gb_r = cpool.tile([1, 2 * D], f32r, tag="gb_r")
    nc.scalar.copy(gb_r[:], gb_row)
    gbbc_ps = psm.tile([128, 2 * D], f32, tag="mm_wide")
    nc.tensor.matmul(gbbc_ps[:], ones_col_r[:], gb_r[:])
    gbbc = cpool.tile([128, 2 * D], f32, tag="gbbc")
    nc.scalar.copy(gbbc[:], gbbc_ps[:])
    seq = cpool.tile([L, D], f32, tag="seq0")
    nc.vector.tensor_tensor(seq[:], text_sb[:], gbbc[:, 0:D], Alu.mult)
    nc.vector.scalar_tensor_tensor(seq[:], seq[:], BN_SCALE, gbbc[:, D : 2 * D], Alu.mult, Alu.add)

    seqT = cpool.tile([128, KD, 128], f32r, tag="seqT0")
    for k in range(KD):
        tp = pst.tile([128, 128], f32, tag="tps")
        nc.tensor.transpose(tp[:], seq[:, k * 128 : (k + 1) * 128], ident[:])
        nc.scalar.copy(seqT[:, k, :], tp[:])

    if _stop(1, seq[0:1, 0:R]):
        return

    # ---------------- one-hot st3[i, t, j] (fills the gram-DMA window) -----
    st3 = cpool.tile([L, NTC, L], bf16, tag="st3")
    nc.vector.tensor_tensor(
        st3[:],
        vT_bf[:, None, :].to_broadcast((L, NTC, L)),
        iota_bf[:, 1:NT, None].to_broadcast((L, NTC, L)),
        Alu.is_equal,
    )

    # ---------------- score2 via gathered Gram table ----------------
    gath = cpool.tile([128, 16 * L], f32, tag="gath")
    nc.gpsimd.ap_gather(
        gath[:], gtab[:], idx_sb[:], channels=128, num_elems=NT2, d=1, num_idxs=16 * L
    )
    mk = cpool.tile([L, L, 16], bf16, tag="mk")
    nc.vector.tensor_tensor(
        mk[:],
        gath[:].rearrange("p (j k) -> p j k", k=16),
        km_bf[:, None, :].to_broadcast((L, L, 16)),
        Alu.mult,
    )
    nc.vector.tensor_tensor(mk[:, :, 0:8], mk[:, :, 0:8], mk[:, :, 8:16], Alu.add)
    nc.vector.tensor_tensor(mk[:, :, 0:4], mk[:, :, 0:4], mk[:, :, 4:8], Alu.add)
    nc.vector.tensor_tensor(mk[:, :, 0:2], mk[:, :, 0:2], mk[:, :, 2:4], Alu.add)
    s2_sb = cpool.tile([L, L], f32, tag="s2_sb")
    nc.vector.tensor_tensor(s2_sb[:], mk[:, :, 0], mk[:, :, 1], Alu.add)

    if _stop(3, s2_sb[0:1, 0:R]):
        return

    # ---------------- EW / bias / mask-pool weights ----------------
    ew_sb = []
    for l in range(3):
        ew = cpool.tile([NTC, D], f32r, tag=f"ew{l}", name=f"ew{l}")
        ewp = psm.tile([NTC, D], f32, tag="mm_wide")
        for k in range(KD):
            nc.tensor.matmul(
                ewp[:], et_sb[:, k, 1:NT], W_sb[l][:, k, :],
                start=(k == 0), stop=(k == KD - 1),
            )
        nc.scalar.copy(ew[:], ewp[:])
        ew_sb.append(ew)

    b_r = cpool.tile([1, 3 * D], f32r, tag="b_r")
    nc.scalar.copy(b_r[:], blobr[0:1, 512:1280])
    b_rows_r = [b_r[0:1, D * l : D * (l + 1)] for l in range(3)]

    m_f = cpool.tile([L, 1], f32, tag="m_f")
    nc.vector.tensor_copy(m_f[:], m_i[:])
    cnt_ps = psm.tile([1, 1], f32, tag="mm_small")
    nc.tensor.matmul(cnt_ps[:], m_f[:], m_f[:])
    rcnt = cpool.tile([1, 1], f32, tag="rcnt")
    nc.vector.tensor_scalar_add(rcnt[:], cnt_ps[:], 1e-10)
    nc.vector.reciprocal(rcnt[:], rcnt[:])

    nmx3 = wpool.tile([1, 1], f32, tag="nmx3")
    nc.vector.tensor_reduce(nmx3[:], ens_sb[:], axis=Axis.X, op=Alu.max, negate=True)
    e3 = wpool.tile([1, 3], f32, tag="e3")
    z3 = wpool.tile([1, 1], f32, tag="z3")
    nc.scalar.activation(e3[:], ens_sb[:], Act.Exp, bias=nmx3[:], scale=1.0, accum_out=z3[:])
    rz3 = wpool.tile([1, 1], f32, tag="rz3")
    nc.vector.reciprocal(rz3[:], z3[:])
    wc = cpool.tile([1, 3], f32, tag="wc")
    nc.vector.tensor_scalar(wc[:], e3[:], rz3[:], rcnt[:], Alu.mult, Alu.mult)
    wbc_ps = psm.tile([128, 3], f32, tag="mm_small")
    nc.tensor.matmul(wbc_ps[:], ones_col[:], wc[:])
    wbc = cpool.tile([128, 3], f32, tag="wbc")
    nc.vector.tensor_copy(wbc[:], wbc_ps[:])
    m_w = cpool.tile([L, 3], f32, tag="m_w")
    nc.vector.tensor_tensor(m_w[:], m_f[:].to_broadcast((L, 3)), wbc[:], Alu.mult)
    if _stop(9, m_w[0:64, 0:1]):
        return

    ct_sb = cpool.tile([NTC, 128], f32r, tag="ct")
    ens_ps = psa.tile([128, KD], f32, tag="ensT", name="ensT")

    if _stop(4, seq[0:1, 0:R]):
        return

    # layer-0 s1 can run long before s2 lands: pre-scale it so the s2-gated
    # op is a single cheap add off PSUM's access-latency path. The layer-0
    # softmax shift M = max(score1) + max|s2| is also precomputed: softmax is
    # shift-invariant, so any bound >= rowmax(score) is exact.
    score1_0 = cpool.tile([L, L], f32, tag="score1_0")
    bias0 = cpool.tile([L, 1], f32, tag="bias0")

    # ---------------- the three TGCN layers ----------------
    for l in range(3):
        s1_ps = psm.tile([L, L], f32, tag="mm_out")
        for k in range(KD):
            nc.tensor.matmul(s1_ps[:], seqT[:, k, :], seqT[:, k, :], start=(k == 0), stop=(k == KD - 1))
        score = wpool.tile([L, L], f32, tag="score")
        if l == 0:
            nc.vector.tensor_scalar(score1_0[:], s1_ps[:], INV_SQRT_D, None, Alu.mult)
            nmx = wpool.tile([L, 1], f32, tag="nmx")
            nc.vector.tensor_reduce(nmx[:], score1_0[:], axis=Axis.X, op=Alu.max, negate=True)
            nc.vector.tensor_scalar_add(bias0[:], nmx[:], -30.0)
            nc.vector.tensor_tensor(score[:], score1_0[:], s2_sb[:], Alu.add)
        else:
            nc.vector.scalar_tensor_tensor(score[:], s1_ps[:], INV_SQRT_D, s2_sb[:], Alu.mult, Alu.add)
            nmx = wpool.tile([L, 1], f32, tag="nmx")
            nc.vector.tensor_reduce(nmx[:], score[:], axis=Axis.X, op=Alu.max, negate=True)
            bias0 = nmx
        e_sb = wpool.tile([L, L], f32, tag="e_sb")
        z = wpool.tile([L, 1], f32, tag="z")
        nc.scalar.activation(e_sb[:], score[:], Act.Exp, bias=bias0[:], scale=1.0, accum_out=z[:])
        rz = wpool.tile([L, 1], f32, tag="rz")
        nc.vector.reciprocal(rz[:], z[:])
        att_bf = wpool.tile([L, L], bf16, tag="att_bf")
        nc.vector.scalar_tensor_tensor(att_bf[:], e_sb[:], rz[:], adj_sb[:], Alu.mult, Alu.mult)

        atT_ps = pst.tile([128, 128], bf16, tag="tps_bf", bufs=1)
        nc.tensor.transpose(atT_ps[:], att_bf[:], ident_bf[:])
        attT = wpool.tile([L, L], f32r, tag="attT")
        nc.scalar.copy(attT[:], atT_ps[:])

        tw_ps = psm.tile([L, D], f32, tag="mm_wide")
        for k in range(KD):
            nc.tensor.matmul(
                tw_ps[:], seqT[:, k, :], W_sb[l][:, k, :],
                start=(k == 0), stop=(k == KD - 1),
            )
        tw = wpool.tile([L, D], f32r, tag="tw")
        nc.scalar.copy(tw[:], tw_ps[:])

        # out1 + bias accumulate early (independent of the C-path)
        out_ps = psm.tile([L, D], f32, tag="mm_out2")
        nc.tensor.matmul(out_ps[:], attT[:], tw[:], start=True, stop=False)
        nc.tensor.matmul(out_ps[:], ones_col_r[:], b_rows_r[l], start=False, stop=False)

        # C[i,t] = sum_j att[i,j] * [v[j,i]==t]: bcast-mult + halving tree
        prod = cpool.tile([L, NTC, L], bf16, tag="prod", name="prod")
        nc.vector.tensor_tensor(
            prod[:], att_bf[:, None, :].to_broadcast((L, NTC, L)), st3[:], Alu.mult
        )
        w = L
        while w > 8:
            h = w // 2
            nc.vector.tensor_tensor(
                prod[:, :, 0:h], prod[:, :, 0:h], prod[:, :, h:w], Alu.add
            )
            w = h
        c_sb = wpool.tile([L, NTC], f32, tag="c_sb")
        nc.vector.tensor_reduce(c_sb[:], prod[:, :, 0:8], axis=Axis.X, op=Alu.add)

        ct_ps = pst.tile([128, 128], f32, tag="tps")
        nc.tensor.transpose(ct_ps[0:NTC, :], c_sb[:], ident[:])
        nc.scalar.copy(ct_sb[:], ct_ps[0:NTC, :])

        nc.tensor.matmul(out_ps[:], ct_sb[:], ew_sb[l][:], start=False, stop=True)

        seq_n = wpool.tile([L, D], f32, tag="seq_n")
        nc.scalar.activation(seq_n[:], out_ps[:], Act.Relu)

        for k in range(KD):
            nc.tensor.matmul(
                ens_ps[:, k : k + 1], seq_n[:, k * 128 : (k + 1) * 128], m_w[:, l : l + 1],
                start=(l == 0), stop=(l == 2),
            )

        if l < 2:
            seqT = wpool.tile([128, KD, 128], f32r, tag="seqT_n")
            for k in range(KD):
                tp = pst.tile([128, 128], f32, tag="tps")
                nc.tensor.transpose(tp[:], seq_n[:, k * 128 : (k + 1) * 128], ident[:])
                if k == 0:
                    nc.scalar.copy(seqT[:, k, :], tp[:])
                else:
                    nc.vector.tensor_copy(seqT[:, k, :], tp[:])
        seq = seq_n
        if _stop(5 + l, seq[0:1, 0:R]):
            return

    # ---------------- final fc ----------------
    ensT = wpool.tile([128, KD, 1], f32, tag="ensT_sb")
    for k in range(KD):
        nc.vector.tensor_copy(ensT[:, k, :], ens_ps[:, k : k + 1])
    if _stop(8, ensT[0:64, 0, :]):
        return
    fin_ps = psm.tile([1, R], f32, tag="mm_small")
    for k in range(KD):
        nc.tensor.matmul(fin_ps[:], ensT[:, k, :], fcw_sb[:, k, :], start=(k == 0), stop=(k == KD - 1))
    out_sb = wpool.tile([1, R], f32, tag="out_sb")
    nc.vector.tensor_tensor(out_sb[:], fin_ps[:], fcb_sb[:], Alu.add)
    nc.sync.dma_start(out_d.ap(), out_sb[:])

    for p in (psa, psm, pst, wpool, cpool):
        p.release()


_NC_CACHE = {}


def build_nc():
    if "nc" not in _NC_CACHE:
        nc = bacc.Bacc("TRN2", target_bir_lowering=False, debug=False)
        with tile.TileContext(nc) as tc:
            _build_graph(nc, tc)
        nc.compile()
        _NC_CACHE["nc"] = nc
    return _NC_CACHE["nc"]


def _pack_w(w):
    """[256, N] -> [128, 2N] in ki-ko-n layout."""
    return np.ascontiguousarray(
        np.asarray(w, np.float32).reshape(KD, 128, -1).transpose(1, 0, 2).reshape(128, -1)
    )


def _in_maps(inputs):
    emb = np.zeros((128, D), np.float32)
    emb[1:NT] = np.asarray(inputs["dep_emb"], np.float32)[1:NT]  # row 0 zeroed
    w_packed = [_pack_w(inputs[f"W{i}"]) for i in (1, 2, 3)]
    fcw_packed = _pack_w(inputs["fc_W"])
    row = np.concatenate(
        [
            np.asarray(inputs["gamma"], np.float32).reshape(-1),
            np.asarray(inputs["beta"], np.float32).reshape(-1),
            np.asarray(inputs["b1"], np.float32).reshape(-1),
            np.asarray(inputs["b2"], np.float32).reshape(-1),
            np.asarray(inputs["b3"], np.float32).reshape(-1),
            np.asarray(inputs["ens_lin"], np.float32).reshape(-1),
            np.asarray(inputs["fc_b"], np.float32).reshape(-1),
        ]
    )[None, :]
    blob_b = np.concatenate(w_packed + [fcw_packed], axis=1)
    maps = []
    for c in range(B):
        blob_e = np.concatenate(
            [
                np.ascontiguousarray(inputs["dep_value"][c], np.int32).view(np.float32),
                np.asarray(inputs["input_mask"][c], np.int32).reshape(128, 1).view(np.float32),
                np.asarray(inputs["text"][c], np.float32),
                np.asarray(inputs["dep_adj"][c], np.float32),
            ],
            axis=1,
        )
        maps.append(
            {
                "blob_m": np.ascontiguousarray(emb, np.float32),
                "blob_e": np.ascontiguousarray(blob_e, np.float32),
                "blob_b": np.ascontiguousarray(blob_b, np.float32),
                "blob_r": np.ascontiguousarray(row, np.float32),
            }
        )
    return maps


def kernel(**inputs):
    nc = build_nc()
    res = run_bass_kernel_spmd(nc, _in_maps(inputs), core_ids=list(range(B)))
    return np.concatenate([r["out"] for r in res.results], axis=0)


def kernel_traced(**inputs):
    """Same as kernel() but returns (output, exec_time_ns)."""
    nc = build_nc()
    res = run_bass_kernel_spmd(
        nc, _in_maps(inputs), core_ids=list(range(B)), trace=True
    )
    out = np.concatenate([r["out"] for r in res.results], axis=0)
    return out, res.exec_time_ns
